# revision 1
# baseline (speedup 1.0000x reference)
"""Trainium2 Bass kernel for a dense transformer block (B=4, T=2048, D=1024, H=16).

Sharding: 8 cores = 4 pairs; pair p owns batch p. Within a pair:
  - attention is head-split (8 heads per core, all 2048 tokens, causal),
  - each core computes partial w_o output (its 512 y-dims x all tokens),
  - one 2-rank ReduceScatter per pair sums the partials and hands rank0
    tokens [0:1024], rank1 tokens [1024:2048],
  - residual + LN2 + MLP are token-split (1024 tokens per core), no further
    communication.
The SPMD program is rank-uniform; all rank differences are carried by data
(weight shards, x_res token slice, RS output placement).
"""

import sys

sys.path.insert(0, "/opt/trn_rl_repo")

import numpy as np
import ml_dtypes

import concourse.bass as bass
import concourse.tile as tile
from concourse import mybir
from concourse.bass_utils import run_bass_kernel_spmd
from concourse.masks import make_identity

BF16 = mybir.dt.bfloat16
F32 = mybir.dt.float32
AF = mybir.ActivationFunctionType

T = 2048  # sequence length
D = 1024  # model dim
H = 16  # total heads
HD = 64  # head dim
HL = 8  # heads per core
P = 128  # partitions
NT = T // P  # 16 token tiles
ND = D // P  # 8 d tiles
TL = T // 2  # tokens owned per core (1024)
NTL = TL // P  # 8
FC = 4 * D  # 4096
NFC = FC // P  # 32
EPS = 1e-5


def _patch_tile_drain():
    """walrus in this container caps sync-wait commands per instruction; the
    TileContext tail drain carries several. Redistribute one wait per NOP."""
    from concourse.tile import ScopedClock

    def patched(self, tick_clock, wait_clock):
        nc = self.nc
        probe = nc.sync.nop(nofuse=True)
        wait_clock.add_sem_waits(probe.ins, ScopedClock({None: tick_clock.global_clock}))
        si = probe.ins.sync_info
        waits = list(si.on_wait) if si and si.on_wait else []
        if len(waits) > 1:
            probe.ins.sync_info = mybir.SyncInfo(
                on_wait=waits[:1], on_update=list(si.on_update or [])
            )
            for i in range(1, len(waits)):
                nop = nc.sync.nop(nofuse=True)
                nop.ins.sync_info = mybir.SyncInfo(on_wait=waits[i : i + 1], on_update=[])
        nc.all_engine_barrier()
        popped = nc._tile_sem_poison_stack.pop()
        assert popped is self._sem_poison
        nc.clear_and_free_semaphores(list(self.sems.allocated().values()))
        nc.all_engine_barrier()

    tile.TileContext._drain_and_barrier = patched


_SYNC_WAIT_CAP = 1


def _split_sync_waits(nc, cap=_SYNC_WAIT_CAP):
    """walrus rejects instructions carrying more than ~2 sync waits; hoist the
    excess onto preceding same-engine NOPs (more conservative ordering, same
    semantics)."""
    cnt = 0
    for f in nc.m.functions:
        for bb in f.blocks:
            new_insts = []
            for inst in bb.instructions:
                si = inst.sync_info
                waits = list(si.on_wait) if si and si.on_wait else []
                if len(waits) > cap:
                    for i in range(0, len(waits) - cap, cap):
                        nop = mybir.InstNoOp(name=f"waitsplit_{cnt}", ins=[], outs=[])
                        cnt += 1
                        nop.engine = inst.engine
                        nop.sync_info = mybir.SyncInfo(
                            on_wait=waits[i : i + cap], on_update=[]
                        )
                        new_insts.append(nop)
                    inst.sync_info = mybir.SyncInfo(
                        on_wait=waits[len(waits) - cap :],
                        on_update=list(si.on_update or []),
                    )
                new_insts.append(inst)
            bb.instructions[:] = new_insts


def _bcast_ap(ap_1d, parts):
    """Partition-broadcast a 1-D DRAM AP to [parts, len]."""
    return bass.AP(
        tensor=ap_1d.tensor,
        offset=ap_1d.offset,
        ap=[[0, parts], list(ap_1d.ap[0])],
    )


def _layernorm(nc, pool, x_tile, g_sb, b_sb, out_bf, eps_sb):
    """LN over free axis (D=1024) of x_tile [128, 1024] f32.
    Writes out_bf [128, 1024] bf16 = (x-mu)*rstd*g + b."""
    stats = pool.tile([P, 2, 6], F32, tag="ln_stats")
    for s in range(2):
        nc.vector.bn_stats(out=stats[:, s, :], in_=x_tile[:, s * 512 : (s + 1) * 512])
    mv = pool.tile([P, 2], F32, tag="ln_mv")
    nc.vector.bn_aggr(out=mv[:], in_=stats[:])
    # rstd = 1/sqrt(var+eps)
    rstd = pool.tile([P, 1], F32, tag="ln_rstd")
    nc.scalar.activation(
        out=rstd[:], in_=mv[:, 1:2], func=AF.Sqrt, bias=eps_sb[:], scale=1.0
    )
    nc.vector.reciprocal(out=rstd[:], in_=rstd[:])
    xn = pool.tile([P, D], F32, tag="ln_xn")
    nc.vector.tensor_scalar(
        out=xn[:],
        in0=x_tile[:],
        scalar1=mv[:, 0:1],
        scalar2=rstd[:],
        op0=mybir.AluOpType.subtract,
        op1=mybir.AluOpType.mult,
    )
    xg = pool.tile([P, D], BF16, tag="ln_xg")
    nc.vector.tensor_mul(out=xg[:], in0=xn[:], in1=g_sb[:])
    nc.vector.tensor_add(out=out_bf[:], in0=xg[:], in1=b_sb[:])


def build_kernel(stop_after="F"):
    nc = bass.Bass()

    x_ext = nc.declare_dram_parameter("x", [T, D], F32, isOutput=False)
    xres_ext = nc.declare_dram_parameter("x_res", [TL, D], F32, isOutput=False)
    wqk_ext = nc.declare_dram_parameter("wqk", [D, D], BF16, isOutput=False)
    wv_ext = nc.declare_dram_parameter("wv", [D, HL * HD], BF16, isOutput=False)
    wo_ext = nc.declare_dram_parameter("wo", [HL * HD, D], BF16, isOutput=False)
    wfc_ext = nc.declare_dram_parameter("wfc", [D, FC], BF16, isOutput=False)
    wproj_ext = nc.declare_dram_parameter("wproj", [FC, D], BF16, isOutput=False)
    ln1g_ext = nc.declare_dram_parameter("ln1g", [D], F32, isOutput=False)
    ln1b_ext = nc.declare_dram_parameter("ln1b", [D], BF16, isOutput=False)
    ln2g_ext = nc.declare_dram_parameter("ln2g", [D], F32, isOutput=False)
    ln2b_ext = nc.declare_dram_parameter("ln2b", [D], BF16, isOutput=False)
    bfc_ext = nc.declare_dram_parameter("bfc", [FC], F32, isOutput=False)
    bproj_ext = nc.declare_dram_parameter("bproj", [D], F32, isOutput=False)
    masks_ext = nc.declare_dram_parameter("masks", [4, P, 512], BF16, isOutput=False)
    out_ext = nc.declare_dram_parameter("out", [TL, D], F32, isOutput=True)

    cc_ins = [nc.dram_tensor(f"cc_in{c}", [512, D], BF16) for c in range(4)]
    cc_outs = [nc.dram_tensor(f"cc_out{c}", [256, D], BF16) for c in range(4)]
    resid_dram = nc.dram_tensor("resid_dram", [TL, D], F32)

    x_r = x_ext.rearrange("(t p) d -> p t d", p=P)
    xres_r = xres_ext.rearrange("(t p) d -> p t d", p=P)
    wqk_r = wqk_ext.rearrange("(dt p) c -> p dt c", p=P)
    wv_r = wv_ext.rearrange("(dt p) c -> p dt c", p=P)
    wo_r = wo_ext.rearrange("(yt p) c -> p yt c", p=P)
    wfc_r = wfc_ext.rearrange("(dt p) c -> p dt c", p=P)
    wproj_r = wproj_ext.rearrange("(ft p) c -> p ft c", p=P)
    masks_r = masks_ext.rearrange("j p q -> p j q")
    cc_in_rs = [t.rearrange("(t p) d -> p t d", p=P) for t in cc_ins]
    cc_out_rs = [t.rearrange("(t p) d -> p t d", p=P) for t in cc_outs]
    resid_r = resid_dram.rearrange("(t p) d -> p t d", p=P)
    out_r = out_ext.rearrange("(t p) d -> p t d", p=P)

    with tile.TileContext(nc) as tc:
        with (
            tc.tile_pool(name="singles", bufs=1) as singles,
            tc.tile_pool(name="psA", bufs=3, space="PSUM") as psA,
            tc.tile_pool(name="psT", bufs=2, space="PSUM") as psT,
            tc.tile_pool(name="psB", bufs=1, space="PSUM") as psB,
            tc.tile_pool(name="ypsum", bufs=2, space="PSUM") as ypsumP,
        ):
            # ---- constants ----
            ident = singles.tile([P, P], BF16)
            make_identity(nc, ident)
            g1_sb = singles.tile([P, D], F32)
            nc.scalar.dma_start(out=g1_sb[:], in_=_bcast_ap(ln1g_ext[:], P))
            b1_sb = singles.tile([P, D], BF16)
            nc.scalar.dma_start(out=b1_sb[:], in_=_bcast_ap(ln1b_ext[:], P))
            g2_sb = singles.tile([P, D], F32)
            nc.scalar.dma_start(out=g2_sb[:], in_=_bcast_ap(ln2g_ext[:], P))
            b2_sb = singles.tile([P, D], BF16)
            nc.scalar.dma_start(out=b2_sb[:], in_=_bcast_ap(ln2b_ext[:], P))
            bproj_sb = singles.tile([P, D], F32)
            nc.scalar.dma_start(out=bproj_sb[:], in_=_bcast_ap(bproj_ext[:], P))
            bfc_sb = singles.tile([P, NFC], F32)
            nc.scalar.dma_start(out=bfc_sb[:], in_=bfc_ext.rearrange("(o p) -> p o", p=P))
            masks_sb = singles.tile([P, 4, 512], BF16)
            nc.scalar.dma_start(out=masks_sb[:], in_=masks_r[:])
            ones1 = singles.tile([1, HD], BF16)
            nc.vector.memset(ones1[:], 1.0)
            eps_sb = singles.tile([P, 1], F32)
            nc.vector.memset(eps_sb[:], EPS)

            hT = singles.tile([P, ND, TL], BF16)  # LN2(resid)^T
            with tc.tile_pool(name="pBC", bufs=1) as pBC:
                qkT = pBC.tile([P, ND, T], BF16)  # [qcols|kcols, tok]
                v_sb = pBC.tile([P, NT, HL, HD + 1], BF16)
                nc.vector.memset(v_sb[:, :, :, HD : HD + 1], 1.0)

                with tc.tile_pool(name="pAB", bufs=1) as pAB:
                    xlT = pAB.tile([P, ND, T], BF16)  # LN1(x)^T  [d, tok]
                    wqk_sb = pAB.tile([P, ND, D], BF16)
                    nc.scalar.dma_start(out=wqk_sb[:], in_=wqk_r[:])
                    wv_sb = pAB.tile([P, ND, HL * HD], BF16)
                    nc.scalar.dma_start(out=wv_sb[:], in_=wv_r[:])

                    # ---- stage A: LN1 + transpose ----
                    with tc.tile_pool(name="sa", bufs=4) as sa:
                        for tt in range(NT):
                            x_tile = sa.tile([P, D], F32, tag="x_tile")
                            nc.sync.dma_start(out=x_tile[:], in_=x_r[:, tt, :])
                            xl = sa.tile([P, D], BF16, tag="xl")
                            _layernorm(nc, sa, x_tile, g1_sb, b1_sb, xl, eps_sb)
                            for dt in range(ND):
                                tp = psT.tile([P, P], BF16, tag="tp")
                                nc.tensor.transpose(
                                    tp[:], xl[:, dt * P : (dt + 1) * P], ident[:]
                                )
                                nc.scalar.copy(
                                    out=xlT[:, dt, tt * P : (tt + 1) * P], in_=tp[:]
                                )

                    # ---- stage B: qkT = (xl @ Wqk)^T and V = xl @ Wv ----
                    for ct in range(ND if stop_after >= "B" else 0):
                        for qc in range(4):
                            qp = psA.tile([P, 512], F32, tag="mm")
                            for dt in range(ND):
                                nc.tensor.matmul(
                                    qp[:],
                                    lhsT=wqk_sb[:, dt, ct * P : (ct + 1) * P],
                                    rhs=xlT[:, dt, qc * 512 : (qc + 1) * 512],
                                    start=(dt == 0),
                                    stop=(dt == ND - 1),
                                )
                            nc.scalar.copy(
                                out=qkT[:, ct, qc * 512 : (qc + 1) * 512], in_=qp[:]
                            )
                    for tt in range(NT if stop_after >= "B" else 0):
                        vp = psA.tile([P, 512], F32, tag="mm")
                        for dt in range(ND):
                            nc.tensor.matmul(
                                vp[:],
                                lhsT=xlT[:, dt, tt * P : (tt + 1) * P],
                                rhs=wv_sb[:, dt, :],
                                start=(dt == 0),
                                stop=(dt == ND - 1),
                            )
                        nc.vector.tensor_copy(
                            out=v_sb[:, tt, :, 0:HD],
                            in_=vp.rearrange("p (h e) -> p h e", h=HL),
                        )

                # ---- stages C/D/RS/E interleaved per q-chunk ----
                with tc.tile_pool(name="pCD", bufs=1) as pCD:
                    yT = pCD.tile([P, HL * HD // P, T], BF16)  # [ydim, tok]
                    wo_sb = pCD.tile([P, HL * HD // P, D], BF16)
                    nc.scalar.dma_start(out=wo_sb[:], in_=wo_r[:])

                    with tc.tile_pool(name="sc", bufs=3) as sc:

                        def emit_post_chain(pqc):
                            # ---- stage D: partial w_o for chunk pqc ----
                            for t4 in range(4):
                                tt = 4 * pqc + t4
                                for half in range(2):
                                    op = psA.tile([P, 512], F32, tag="mm")
                                    for yt in range(HL * HD // P):
                                        nc.tensor.matmul(
                                            op[:],
                                            lhsT=yT[:, yt, tt * P : (tt + 1) * P],
                                            rhs=wo_sb[:, yt, half * 512 : (half + 1) * 512],
                                            start=(yt == 0),
                                            stop=(yt == HL * HD // P - 1),
                                        )
                                    ob = sc.tile([P, 512], BF16, tag="ob")
                                    nc.scalar.copy(out=ob[:], in_=op[:])
                                    nc.sync.dma_start(
                                        out=cc_in_rs[pqc][:, t4, half * 512 : (half + 1) * 512],
                                        in_=ob[:],
                                    )
                            # ---- RS for chunk pqc ----
                            nc.gpsimd.collective_compute(
                                "ReduceScatter",
                                mybir.AluOpType.add,
                                ins=[cc_ins[pqc][:]],
                                outs=[cc_outs[pqc][:]],
                                replica_groups=[[0, 1], [2, 3], [4, 5], [6, 7]],
                            )
                            # ---- stage E: residual + LN2 + transpose (2 tiles) ----
                            if stop_after < "E":
                                return
                            for i2 in range(2):
                                tt = 2 * pqc + i2
                                rs_bf = sc.tile([P, D], BF16, tag="rs_bf")
                                nc.sync.dma_start(
                                    out=rs_bf[:], in_=cc_out_rs[pqc][:, i2, :]
                                )
                                xr = sc.tile([P, D], F32, tag="xr")
                                nc.sync.dma_start(out=xr[:], in_=xres_r[:, tt, :])
                                rs_f = sc.tile([P, D], F32, tag="rs_f")
                                nc.vector.tensor_copy(out=rs_f[:], in_=rs_bf[:])
                                resid_t = sc.tile([P, D], F32, tag="resid_t")
                                nc.vector.tensor_add(
                                    out=resid_t[:], in0=xr[:], in1=rs_f[:]
                                )
                                nc.sync.dma_start(out=resid_r[:, tt, :], in_=resid_t[:])
                                h2 = sc.tile([P, D], BF16, tag="h2")
                                _layernorm(nc, sc, resid_t, g2_sb, b2_sb, h2, eps_sb)
                                for dt in range(ND):
                                    tp2 = psT.tile([P, P], BF16, tag="tp")
                                    nc.tensor.transpose(
                                        tp2[:], h2[:, dt * P : (dt + 1) * P], ident[:]
                                    )
                                    nc.scalar.copy(
                                        out=hT[:, dt, tt * P : (tt + 1) * P], in_=tp2[:]
                                    )

                        for qc in range(4 if stop_after >= "C" else 0):
                            nkt = 4 * (qc + 1)
                            for h in range(HL):
                                po = (h % 2) * HD
                                qt_ct = h // 2
                                kt_ct = 4 + h // 2
                                yp = ypsumP.tile([HD + 1, 512], F32, tag="yp")
                                for kt in range(nkt):
                                    sp = psA.tile([P, 512], F32, tag="mm")
                                    nc.tensor.matmul(
                                        sp[:],
                                        lhsT=qkT[po : po + HD, kt_ct, kt * P : (kt + 1) * P],
                                        rhs=qkT[po : po + HD, qt_ct, qc * 512 : (qc + 1) * 512],
                                        start=True,
                                        stop=True,
                                    )
                                    pt = sc.tile([P, 512], BF16, tag="pt")
                                    nc.scalar.activation(
                                        out=pt[:], in_=sp[:], func=AF.Exp, scale=0.125
                                    )
                                    j = kt - 4 * qc
                                    if j >= 0:
                                        nc.vector.tensor_mul(
                                            out=pt[:], in0=pt[:], in1=masks_sb[:, j, :]
                                        )
                                    nc.tensor.matmul(
                                        yp[:],
                                        lhsT=v_sb[:, kt, h, :],
                                        rhs=pt[:],
                                        start=(kt == 0),
                                        stop=(kt == nkt - 1),
                                    )
                                rec = sc.tile([1, 512], F32, tag="rec")
                                nc.vector.reciprocal(out=rec[:], in_=yp[HD : HD + 1, :])
                                recb = sc.tile([1, 512], BF16, tag="recb")
                                nc.vector.tensor_copy(out=recb[:], in_=rec[:])
                                bp = psB.tile([HD, 512], F32, tag="bp")
                                nc.tensor.matmul(
                                    bp[:], lhsT=ones1[:], rhs=recb[:], start=True, stop=True
                                )
                                bps = sc.tile([HD, 512], F32, tag="bps")
                                nc.scalar.copy(out=bps[:], in_=bp[:])
                                nc.vector.tensor_mul(
                                    out=yT[po : po + HD, h // 2, qc * 512 : (qc + 1) * 512],
                                    in0=yp[0:HD, :],
                                    in1=bps[:],
                                )

                            if stop_after < "D" or qc == 0:
                                continue
                            # post-chain for the PREVIOUS chunk (pipeline: overlaps
                            # this chunk's attention on PE)
                            emit_post_chain(qc - 1)
                        if stop_after >= "D":
                            emit_post_chain(3)

            # ---- stage F: MLP ----
            with (
                tc.tile_pool(name="pw", bufs=1) as pw,
                tc.tile_pool(name="sf", bufs=3) as sf,
            ):
                wproj_sb = pw.tile([P, NFC, D], BF16)
                nc.scalar.dma_start(out=wproj_sb[:], in_=wproj_r[:])
                for tc2 in range(2 if stop_after >= "F" else 0):
                    gT = sf.tile([P, NFC, 512], BF16, tag="gT", bufs=1)
                    for fct in range(NFC):
                        wfc_tile = sf.tile([P, ND, P], BF16, tag="wfc_tile", bufs=6)
                        nc.sync.dma_start(
                            out=wfc_tile[:], in_=wfc_r[:, :, fct * P : (fct + 1) * P]
                        )
                        fp = psA.tile([P, 512], F32, tag="mm")
                        for dt in range(ND):
                            nc.tensor.matmul(
                                fp[:],
                                lhsT=wfc_tile[:, dt, :],
                                rhs=hT[:, dt, tc2 * 512 : (tc2 + 1) * 512],
                                start=(dt == 0),
                                stop=(dt == ND - 1),
                            )
                        nc.scalar.activation(
                            out=gT[:, fct, :],
                            in_=fp[:],
                            func=AF.Gelu_apprx_tanh,
                            bias=bfc_sb[:, fct : fct + 1],
                            scale=1.0,
                        )
                    for t4 in range(4):
                        tt = tc2 * 4 + t4
                        for half in range(2):
                            pp = psA.tile([P, 512], F32, tag="mm")
                            for fct in range(NFC):
                                nc.tensor.matmul(
                                    pp[:],
                                    lhsT=gT[:, fct, t4 * P : (t4 + 1) * P],
                                    rhs=wproj_sb[:, fct, half * 512 : (half + 1) * 512],
                                    start=(fct == 0),
                                    stop=(fct == NFC - 1),
                                )
                            rt = sf.tile([P, 512], F32, tag="rt")
                            nc.sync.dma_start(
                                out=rt[:],
                                in_=resid_r[:, tt, half * 512 : (half + 1) * 512],
                            )
                            s1 = sf.tile([P, 512], F32, tag="s1")
                            nc.vector.tensor_add(out=s1[:], in0=pp[:], in1=rt[:])
                            o1 = sf.tile([P, 512], F32, tag="o1")
                            nc.vector.tensor_add(
                                out=o1[:],
                                in0=s1[:],
                                in1=bproj_sb[:, half * 512 : (half + 1) * 512],
                            )
                            nc.sync.dma_start(
                                out=out_r[:, tt, half * 512 : (half + 1) * 512],
                                in_=o1[:],
                            )
    _split_sync_waits(nc)
    return nc


_NC_CACHE = None


def _get_nc():
    global _NC_CACHE
    if _NC_CACHE is None:
        _patch_tile_drain()
        _NC_CACHE = build_kernel()
    return _NC_CACHE


def make_in_maps(x, w_attn, w_o, ln1_g, ln1_b, ln2_g, ln2_b, w_fc, b_fc, w_proj, b_proj):
    bf = ml_dtypes.bfloat16
    B = x.shape[0]
    # causal masks for the 4 diagonal-block offsets (k-tile j within a 512-q chunk)
    q_idx = np.arange(512)[None, :]
    k_idx = np.arange(P)[:, None]
    masks = np.stack(
        [(q_idx >= k_idx + P * j).astype(np.float32) for j in range(4)]
    ).astype(bf)

    wq = w_attn[:, :D]  # [D, D] q cols
    wk = w_attn[:, D : 2 * D]
    wv = w_attn[:, 2 * D :]

    in_maps = []
    for core in range(8):
        p, r = core // 2, core % 2
        hs = r * HL * HD  # head-col offset (512)
        wqk = np.concatenate([wq[:, hs : hs + 512], wk[:, hs : hs + 512]], axis=1)
        in_maps.append(
            {
                "x": np.ascontiguousarray(x[p], np.float32),
                "x_res": np.ascontiguousarray(
                    np.concatenate(
                        [
                            x[p, 512 * c + 256 * r : 512 * c + 256 * r + 256]
                            for c in range(4)
                        ],
                        axis=0,
                    ),
                    np.float32,
                ),
                "wqk": np.ascontiguousarray(wqk).astype(bf),
                "wv": np.ascontiguousarray(wv[:, hs : hs + 512]).astype(bf),
                "wo": np.ascontiguousarray(w_o[hs : hs + 512, :]).astype(bf),
                "wfc": np.ascontiguousarray(w_fc).astype(bf),
                "wproj": np.ascontiguousarray(w_proj).astype(bf),
                "ln1g": np.ascontiguousarray(ln1_g, np.float32),
                "ln1b": np.ascontiguousarray(ln1_b).astype(bf),
                "ln2g": np.ascontiguousarray(ln2_g, np.float32),
                "ln2b": np.ascontiguousarray(ln2_b).astype(bf),
                "bfc": np.ascontiguousarray(b_fc, np.float32),
                "bproj": np.ascontiguousarray(b_proj, np.float32),
                "masks": masks,
            }
        )
    return in_maps


def kernel(**inputs):
    inputs = {k: np.asarray(v) for k, v in inputs.items()}
    nc = _get_nc()
    in_maps = make_in_maps(**inputs)
    res = run_bass_kernel_spmd(nc, in_maps, core_ids=list(range(8)))
    x = inputs["x"]
    B = x.shape[0]
    out = np.empty((B, T, D), np.float32)
    for core in range(8):
        p, r = core // 2, core % 2
        o = res.results[core]["out"]
        for c in range(4):
            out[p, 512 * c + 256 * r : 512 * c + 256 * r + 256] = o[
                c * 256 : (c + 1) * 256
            ]
    return out


if __name__ == "__main__":
    rng = np.random.default_rng(0)
    print("building...")
    nc = _get_nc()
    print("built")

